# revision 1
# baseline (speedup 1.0000x reference)
"""3-layer MLP (dense_mlp) Trainium2 Bass kernel.

Reference computation (fp32):
    h1  = relu(x @ w1 + b1)     x: [4096, 2048], w1: [2048, 4096]
    h2  = relu(h1 @ w2 + b2)    w2: [4096, 4096]
    out = h2 @ w3 + b3          w3: [4096, 1000]

Strategy: pure data-parallel over the batch across 8 NeuronCores (512
rows each, weights replicated, no collectives). Matmuls run in fp32r
(TF32) — full-rate on the PE with ~1e-4 relative error.

Inside a core the activations live in transposed [feature, batch]
layout so each layer is psum[f, b] += W[k, f].T @ actT[k, b]: the
weight tile is the stationary operand and the bias is a per-partition
scalar folded into the ScalarE relu(psum + b) evaluation. The host
pre-transposes x / post-transposes the logits (cheap numpy) so the
device does no layout work at all.

The DMA ceiling for 4KB-per-partition descriptor lines measured only
~200 GB/s/core, below the ~300 GB/s the weight stream needs to stay
compute-bound. Weights are therefore pre-packed on the host so that
each weight DMA is a 2MB transfer with 16KB contiguous per partition:
w_packed[kk, fg, p, s, :] = W[(4*kk+s)*128 + p, fg*1024 : (fg+1)*1024]
and the kernel loads [128, 4, 1024] blocks (4 K-tiles x 8 F-tiles).
"""

import os

import numpy as np
import ml_dtypes

import concourse.bass as bass
import concourse.mybir as mybir
import concourse.tile as tile
from concourse import bacc
from concourse.bass_utils import run_bass_kernel_spmd

P = 128
N_CORES = 8
B_TOTAL = 4096
B = B_TOTAL // N_CORES  # per-core batch rows
D0, D1, D2 = 2048, 4096, 4096
D3_RAW, D3 = 1000, 1024  # classifier dim padded to a multiple of 128

FW = 1024       # f-columns per psum group (8 tiles x 128)
FW3 = 512       # layer-3 group width: 2 groups so stores overlap matmuls
KS = 4          # K-tiles packed per weight DMA (16KB/partition lines)
FG = FW // P    # f-tiles per group = 8 (uses all 8 psum banks)

f32 = mybir.dt.float32
bf16 = mybir.dt.bfloat16


def _act_dt(mode):
    if mode == "bf16":
        return bf16
    if mode == "f32r":
        return mybir.dt.float32r
    return f32


def build_nc(mode: str = "f32r") -> bass.Bass:
    """Build the per-core Bass module. `mode` selects the matmul dtype:
    'f32r' (single-pass TF32), 'f32' (two-pass fp32), 'bf16'."""
    K0, K1, K2 = D0 // P, D1 // P, D2 // P
    F1, F2, F3 = D1 // P, D2 // P, D3 // P
    act_dt = _act_dt(mode)

    nc = bacc.Bacc("TRN2", target_bir_lowering=False, name="mlp3")
    xT = nc.dram_tensor("xT", [P, K0, B], act_dt, kind="ExternalInput")
    w1 = nc.dram_tensor("w1", [K0 // KS, F1 // FG, P, KS, FW], act_dt,
                        kind="ExternalInput")
    b1 = nc.dram_tensor("b1", [P, F1], f32, kind="ExternalInput")
    w2 = nc.dram_tensor("w2", [K1 // KS, F2 // FG, P, KS, FW], act_dt,
                        kind="ExternalInput")
    b2 = nc.dram_tensor("b2", [P, F2], f32, kind="ExternalInput")
    w3 = nc.dram_tensor("w3", [K2 // KS, F3 // FG, P, KS, FW], act_dt,
                        kind="ExternalInput")
    b3 = nc.dram_tensor("b3", [P, F3], f32, kind="ExternalInput")
    out = nc.dram_tensor("out", [P, F3, B], f32, kind="ExternalOutput")

    with tile.TileContext(nc) as tc:
        consts = tc.alloc_tile_pool(name="consts", bufs=1, side="left")
        b1_sb = consts.tile([P, F1], f32, name="b1_sb")
        b2_sb = consts.tile([P, F2], f32, name="b2_sb")
        b3_sb = consts.tile([P, F3], f32, name="b3_sb")
        nc.scalar.dma_start(b1_sb, b1[:, :])
        nc.scalar.dma_start(b2_sb, b2[:, :])
        nc.scalar.dma_start(b3_sb, b3[:, :])

        p_xT = tc.alloc_tile_pool(name="xT", bufs=1, side="left")
        xT_sb = p_xT.tile([P, K0, B], act_dt, name="xT_sb")
        # chunk the input load per k-tile (on the ACT HWDGE ring, so the
        # weight stream on the SP ring is not delayed behind it)
        for k in range(K0):
            nc.scalar.dma_start(xT_sb[:, k, :], xT[:, k, :])

        wpool = tc.alloc_tile_pool(name="w", bufs=3, side="right")
        mmps = tc.alloc_tile_pool(name="mmpsum", bufs=8, space="PSUM")

        # HAM warmup: throwaway f32 matmuls with no DMA dependency keep
        # the PE busy from ~7us until the first weight block lands, so the
        # clock gate is at 8/8 when the real stream starts
        warm = consts.tile([P, P], f32, name="warm")
        nc.gpsimd.memset(warm, 1.0)
        wps = mmps.tile([P, B], f32, name="wps", tag="ps")
        for i in range(56):
            nc.tensor.matmul(wps[:, :P], warm, warm,
                             start=(i == 0), stop=(i == 55))

        def layer(actT, w_dram, bias_sb, outT, n_k, n_f, relu,
                  store_to=None, spool=None, fw=FW):
            fgl = fw // P
            for fg in range(n_f // fgl):
                psums = [
                    mmps.tile([P, B], f32, name=f"ps{f}", tag="ps")
                    for f in range(fgl)
                ]
                for kk in range(n_k // KS):
                    wt = wpool.tile([P, KS, fw], act_dt, name="wt", tag="wt")
                    nc.sync.dma_start(wt, w_dram[kk, fg])
                    for s in range(KS):
                        k = kk * KS + s
                        for f in range(fgl):
                            nc.tensor.matmul(
                                psums[f],
                                wt[:, s, f * P:(f + 1) * P],
                                actT[:, k, :],
                                start=(k == 0),
                                stop=(k == n_k - 1),
                            )
                for f in range(fgl):
                    fi = fg * fgl + f
                    if relu:
                        nc.scalar.activation(
                            outT[:, fi, :],
                            psums[f],
                            mybir.ActivationFunctionType.Relu,
                            bias=bias_sb[:, fi:fi + 1],
                            scale=1.0,
                        )
                    else:
                        # final layer: bias-add into a small staging tile and
                        # stream the store so it overlaps remaining matmuls
                        ot = spool.tile([P, B], f32, name="ot", tag="ot")
                        nc.vector.tensor_tensor(
                            ot,
                            psums[f],
                            bias_sb[:, fi:fi + 1].to_broadcast((P, B)),
                            mybir.AluOpType.add,
                        )
                        nc.scalar.dma_start(store_to[:, fi, :], ot)

        p_h1 = tc.alloc_tile_pool(name="h1", bufs=1, side="right")
        h1T = p_h1.tile([P, K1, B], act_dt, name="h1T")
        layer(xT_sb, w1, b1_sb, h1T, K0, F1, True)
        p_xT.release()

        p_h2 = tc.alloc_tile_pool(name="h2", bufs=1, side="left")
        h2T = p_h2.tile([P, K2, B], act_dt, name="h2T")
        layer(h1T, w2, b2_sb, h2T, K1, F2, True)
        p_h1.release()

        p_oT = tc.alloc_tile_pool(name="oT", bufs=3, side="right")
        layer(h2T, w3, b3_sb, None, K2, F3, False,
              store_to=out, spool=p_oT)
        p_h2.release()
        mmps.release()
        p_oT.release()
        wpool.release()
        consts.release()
    nc.compile()
    return nc


def _pack_weights(w: np.ndarray, np_dt, fw=FW) -> np.ndarray:
    """[d_in, d_out] -> [K/KS, d_out/fw, P, KS, fw] so one [128, KS, fw]
    DMA block reads KS*fw*4 bytes contiguous per partition."""
    d_in, d_out = w.shape
    K, F = d_in // P, d_out // fw
    v = w.reshape(K // KS, KS, P, F, fw)
    return np.ascontiguousarray(v.transpose(0, 3, 2, 1, 4)).astype(np_dt)


LAST_RESULT = None  # BassKernelResults of the most recent run (for test.py)


def _ensure_axon_ntff_hook():
    """Register the NTFF-profile hook that bass_utils expects under axon.
    The agent image's antenv lacks axon_hooks; synthesize it from the
    slim ctypes shim in trn_agent_boot. Only needed for trace runs."""
    import sys
    import types

    try:
        from antenv.axon_hooks import get_axon_ntff_profile_hook  # noqa: F401
        return
    except ImportError:
        pass
    try:
        import antenv
        from trn_agent_boot.trn_boot import _ntff_profile_via_ctypes

        hook = _ntff_profile_via_ctypes("/opt/axon/libaxon_pjrt.so")
        mod = types.ModuleType("antenv.axon_hooks")
        state = {"hook": hook}
        mod.get_axon_ntff_profile_hook = lambda: state["hook"]
        mod.set_axon_ntff_profile_hook = lambda h: state.update(hook=h)
        sys.modules["antenv.axon_hooks"] = mod
        antenv.axon_hooks = mod
    except Exception as e:  # degrade to untraced run
        print(f"ntff hook setup failed ({e!r}); tracing disabled")


def kernel(x, w1, b1, w2, b2, w3, b3):
    global LAST_RESULT
    os.environ.setdefault("JAX_PLATFORMS", "axon")
    mode = os.environ.get("KERNEL_MM_MODE", "f32r")
    trace = os.environ.get("KERNEL_TRACE", "0") == "1"
    if trace:
        _ensure_axon_ntff_hook()

    x = np.asarray(x, dtype=np.float32)
    b1 = np.asarray(b1, dtype=np.float32)
    b2 = np.asarray(b2, dtype=np.float32)
    b3 = np.asarray(b3, dtype=np.float32)

    w3f = np.zeros((D2, D3), dtype=np.float32)
    w3f[:, :D3_RAW] = np.asarray(w3, dtype=np.float32)
    b3f = np.zeros((D3,), dtype=np.float32)
    b3f[:D3_RAW] = b3

    np_dt = ml_dtypes.bfloat16 if mode == "bf16" else np.float32
    w1p = _pack_weights(np.asarray(w1, dtype=np.float32), np_dt)
    w2p = _pack_weights(np.asarray(w2, dtype=np.float32), np_dt)
    w3p = _pack_weights(w3f, np_dt)
    b1p = np.ascontiguousarray(b1.reshape(D1 // P, P).T)
    b2p = np.ascontiguousarray(b2.reshape(D2 // P, P).T)
    b3p = np.ascontiguousarray(b3f.reshape(D3 // P, P).T)

    nc = build_nc(mode=mode)
    K0 = D0 // P
    in_maps = []
    for c in range(N_CORES):
        xs = x[c * B:(c + 1) * B]  # [B, D0]
        # xT[p, k, b] = x[b, k*128 + p]
        xT = np.ascontiguousarray(
            xs.reshape(B, K0, P).transpose(2, 1, 0)).astype(np_dt)
        in_maps.append({
            "xT": xT,
            "w1": w1p, "b1": b1p,
            "w2": w2p, "b2": b2p,
            "w3": w3p, "b3": b3p,
        })

    res = run_bass_kernel_spmd(
        nc, in_maps, core_ids=list(range(N_CORES)), trace=trace
    )
    LAST_RESULT = res
    outs = []
    for r in res.results:
        oT = r["out"]  # [P, F3, B]; logits[b, fg*128+p] = oT[p, fg, b]
        outs.append(oT.transpose(2, 1, 0).reshape(B, D3))
    out = np.concatenate(outs, axis=0)
    return np.ascontiguousarray(out[:, :D3_RAW].astype(np.float32))

